# revision 2
# baseline (speedup 1.0000x reference)
"""ConvLSTM cell kernel for Trainium2 (8 NeuronCores).

Sharding: data-parallel over batch B=4 x spatial split of H=64 into 2 halves
(8 shards). The recurrence prevents sharding T. Each core computes its half
with a shrinking row margin (47-t rows at step t) so no cross-core
communication is ever needed: row validity shrinks by 1 per conv step, and
16 margin rows cover all 16 steps. Bottom halves are row-flipped on the host
(x rows flipped + conv kernel dy-flipped) so a single SPMD program serves
all 8 cores.

On-core layout:
  h lives in SBUF as [128, 49, 66] bf16 "HB": partitions 0-63 hold hpad
  (1 zero pad row on top, zero pad cols left/right), partitions 64-127 hold
  the same data shifted down one row. A 3x3 conv then needs only 6 matmul
  issues per 128-wide oc tile: 3 K=128 issues cover tap pairs (dy=0,dy=1)
  for dx=0..2, and 3 K=64 issues cover dy=2. x_t is added in PSUM with an
  identity matmul. Gates use only tanh (sigmoid(z)=(1+tanh(z/2))/2, the 0.5
  pre-scale applied via the ACT scale input, per-partition for the [g;o]
  tile), and the state update runs as 4 fused scalar_tensor_tensor DVE ops
  carrying scaled states C'=2c and Hs=2h (conv weights pre-halved on host to
  compensate; the final output is multiplied by 0.5 on the host, exact in
  fp32).
"""

import sys

sys.path.insert(0, "/opt/trn_rl_repo")

import numpy as np
from ml_dtypes import bfloat16

HIDDEN = 64
T_STEPS = 16
B = 4
H = 64
W = 64
OC = 4 * HIDDEN  # 256
ROWS = 48        # per-core x rows (32 owned + 16 margin)
OWN = 32
WP = W + 2       # padded row width 66
HROWS = ROWS + 1  # hpad rows (1 zero row on top)

_CACHE = {}


def _build_nc():
    from concourse import bacc, mybir
    from concourse.tile import TileContext

    dt = mybir.dt
    Alu = mybir.AluOpType
    Act = mybir.ActivationFunctionType

    nc = bacc.Bacc(None, target_bir_lowering=False)

    x_in = nc.dram_tensor("x", [T_STEPS, 2, 128, ROWS * W], dt.bfloat16,
                          kind="ExternalInput")
    wp_in = nc.dram_tensor("wp", [128, 6 * 128], dt.bfloat16,
                           kind="ExternalInput")
    ws_in = nc.dram_tensor("ws", [64, 6 * 128], dt.bfloat16,
                           kind="ExternalInput")
    id_in = nc.dram_tensor("ident", [128, 128], dt.bfloat16,
                           kind="ExternalInput")
    sc_in = nc.dram_tensor("scale2", [128, 1], dt.float32,
                           kind="ExternalInput")
    hout = nc.dram_tensor("hout", [T_STEPS, 64, OWN * W], dt.bfloat16,
                          kind="ExternalOutput")

    with TileContext(nc) as tc:
        with (
            tc.tile_pool(name="const", bufs=1) as cpool,
            tc.tile_pool(name="state", bufs=1) as spool,
            tc.tile_pool(name="xload", bufs=3) as xpool,
            tc.tile_pool(name="work", bufs=3) as wpool,
            tc.tile_pool(name="ps", bufs=2, space="PSUM") as psp,
        ):
            wp_sb = cpool.tile([128, 6 * 128], dt.bfloat16, tag="wp")
            ws_sb = cpool.tile([64, 6 * 128], dt.bfloat16, tag="ws")
            id_sb = cpool.tile([128, 128], dt.bfloat16, tag="id")
            sc_sb = cpool.tile([128, 1], dt.float32, tag="sc")
            nc.sync.dma_start(out=wp_sb[:], in_=wp_in[:])
            nc.sync.dma_start(out=ws_sb[:], in_=ws_in[:])
            nc.sync.dma_start(out=id_sb[:], in_=id_in[:])
            nc.sync.dma_start(out=sc_sb[:], in_=sc_in[:])

            # h buffers (ping-pong across steps) and cell state C' = 2c
            hb = [
                spool.tile([128, HROWS, WP], dt.bfloat16, tag="hb0", name="hb0"),
                spool.tile([128, HROWS, WP], dt.bfloat16, tag="hb1", name="hb1"),
            ]
            cs = spool.tile([128, ROWS * W], dt.bfloat16, tag="cs")
            nc.vector.memset(hb[0][:], 0.0)
            nc.vector.memset(hb[1][:], 0.0)
            nc.vector.memset(cs[:], 0.0)

            for t in range(T_STEPS):
                R = 47 - t  # output rows this step
                hbr = hb[t % 2]
                hbw = hb[(t + 1) % 2]

                xt = []
                for half in range(2):
                    xti = xpool.tile([128, ROWS * W], dt.bfloat16,
                                     tag=f"x{half}", name=f"x{half}")
                    nc.sync.dma_start(out=xti[:, : R * W],
                                      in_=x_in[t, half][:, : R * W])
                    xt.append(xti)

                nblk = (R + 15) // 16
                for bi in range(nblk):
                    y0 = bi * 16
                    rows = min(16, R - y0)
                    Nb = rows * W

                    tmp = [psp.tile([128, 1024], dt.float32, tag="tmp0", name="tmp0"),
                           psp.tile([128, 1024], dt.float32, tag="tmp1", name="tmp1")]
                    for tau in range(2):
                        nsub = (rows + 7) // 8
                        for sub in range(nsub):
                            yy = y0 + sub * 8
                            sr = min(8, rows - sub * 8)
                            n = sr * W
                            out_ap = tmp[tau][:, sub * 512: sub * 512 + n]
                            first = True
                            if t > 0:
                                for d in range(3):
                                    nc.tensor.matmul(
                                        out_ap,
                                        lhsT=wp_sb[:, (tau * 3 + d) * 128:
                                                   (tau * 3 + d + 1) * 128],
                                        rhs=hbr[:, yy: yy + sr, d: d + W],
                                        start=first, stop=False)
                                    first = False
                                for d in range(3):
                                    nc.tensor.matmul(
                                        out_ap,
                                        lhsT=ws_sb[:, (tau * 3 + d) * 128:
                                                   (tau * 3 + d + 1) * 128],
                                        rhs=hbr[0:64, yy + 2: yy + 2 + sr,
                                                d: d + W],
                                        start=False, stop=False)
                            nc.tensor.matmul(
                                out_ap, lhsT=id_sb[:],
                                rhs=xt[tau][:, yy * W: yy * W + n],
                                start=first, stop=True)

                    # gates: tile0 = [i;f] (both tanh(0.5 z)),
                    # tile1 = [g;o] (g: tanh(z), o: tanh(0.5 z) via scale AP)
                    sif = wpool.tile([128, 1024], dt.bfloat16, tag="sif")
                    sgo = wpool.tile([128, 1024], dt.bfloat16, tag="sgo")
                    nc.scalar.activation(sif[:, :Nb], tmp[0][:, :Nb],
                                         Act.Tanh, scale=0.5)
                    nc.scalar.activation(sgo[:, :Nb], tmp[1][:, :Nb],
                                         Act.Tanh, scale=sc_sb[:])

                    # u = (s_f+1)*C', v = (s_i+1)*g,
                    # C'_new = 0.5*u + v, tc = tanh(0.5*C'_new)
                    # Hs = (s_o+1)*tc  (written into hbw rows, = 2h)
                    u = wpool.tile([128, 1024], dt.bfloat16, tag="u")
                    v = wpool.tile([128, 1024], dt.bfloat16, tag="v")
                    tch = wpool.tile([128, 1024], dt.bfloat16, tag="tch")
                    cseg = cs[64:128, y0 * W: y0 * W + Nb]
                    nc.vector.scalar_tensor_tensor(
                        u[64:128, :Nb], sif[64:128, :Nb], 1.0, cseg,
                        Alu.add, Alu.mult)
                    nc.vector.scalar_tensor_tensor(
                        v[64:128, :Nb], sif[0:64, :Nb], 1.0, sgo[0:64, :Nb],
                        Alu.add, Alu.mult)
                    nc.vector.scalar_tensor_tensor(
                        cseg, u[64:128, :Nb], 0.5, v[64:128, :Nb],
                        Alu.mult, Alu.add)
                    nc.scalar.activation(tch[64:128, :Nb], cseg,
                                         Act.Tanh, scale=0.5)
                    nc.vector.scalar_tensor_tensor(
                        hbw[0:64, 1 + y0: 1 + y0 + rows, 1: 1 + W],
                        sgo[64:128, :Nb], 1.0, tch[64:128, :Nb],
                        Alu.add, Alu.mult)

                    # shifted copy for the next step's K=128 tap pairs:
                    # upper[r] = lower[r+1]
                    if t < T_STEPS - 1:
                        nc.vector.tensor_copy(
                            hbw[64:128, y0: y0 + rows, :],
                            hbw[0:64, y0 + 1: y0 + 1 + rows, :])

                # store owned rows (hpad rows 1..32) of Hs = 2h
                nc.scalar.dma_start(out=hout[t],
                                    in_=hbw[0:64, 1: 1 + OWN, 1: 1 + W])

    nc.finalize()
    return nc


def _prep_inputs(x, w_h2h):
    """Build per-core input maps. Cores: core = b*2 + half."""
    # gate order in PSUM tiles: tile0 = [i(0:64); f(64:128)],
    # tile1 = [g(192:256); o(128:192)]
    perm = np.concatenate([np.arange(0, 64), np.arange(64, 128),
                           np.arange(192, 256), np.arange(128, 192)])
    w_eff = (w_h2h.astype(np.float32) * 0.5)[perm]  # [256, 64, 3, 3], /2 for Hs=2h

    def pack_w(weff):
        wp = np.zeros((128, 2, 3, 128), np.float32)
        ws = np.zeros((64, 2, 3, 128), np.float32)
        for tau in range(2):
            blk = weff[tau * 128: (tau + 1) * 128]  # [128oc, 64ic, 3, 3]
            for d in range(3):
                wp[0:64, tau, d, :] = blk[:, :, 0, d].T
                wp[64:128, tau, d, :] = blk[:, :, 1, d].T
                ws[:, tau, d, :] = blk[:, :, 2, d].T
        return (wp.reshape(128, 6 * 128).astype(bfloat16),
                ws.reshape(64, 6 * 128).astype(bfloat16))

    wp_top, ws_top = pack_w(w_eff)
    wp_bot, ws_bot = pack_w(w_eff[:, :, ::-1, :])
    ident = np.eye(128, dtype=np.float32).astype(bfloat16)
    scale2 = np.concatenate([np.ones((64, 1), np.float32),
                             np.full((64, 1), 0.5, np.float32)])

    xp = x[:, :, perm]  # [T, B, 256, H, W] permuted channels
    in_maps = []
    for b in range(B):
        for half in range(2):
            if half == 0:
                xs = xp[:, b, :, 0:ROWS, :]
            else:
                xs = xp[:, b, :, H - ROWS:, :][:, :, ::-1, :]
            xs = np.ascontiguousarray(xs).astype(bfloat16)
            xs = xs.reshape(T_STEPS, 2, 128, ROWS * W)
            in_maps.append({
                "x": xs,
                "wp": wp_top if half == 0 else wp_bot,
                "ws": ws_top if half == 0 else ws_bot,
                "ident": ident,
                "scale2": scale2,
            })
    return in_maps


def kernel(x, w_h2h):
    from concourse import bass_utils

    if "nc" not in _CACHE:
        _CACHE["nc"] = _build_nc()
    nc = _CACHE["nc"]

    in_maps = _prep_inputs(np.asarray(x), np.asarray(w_h2h))
    res = bass_utils.run_bass_kernel_spmd(nc, in_maps,
                                          core_ids=list(range(8)),
                                          **_CACHE.get("run_kwargs", {}))
    _CACHE["last_results"] = res

    out = np.zeros((T_STEPS, B, HIDDEN, H, W), np.float32)
    for b in range(B):
        for half in range(2):
            core = b * 2 + half
            hs = res.results[core]["hout"].astype(np.float32) * 0.5
            hs = hs.reshape(T_STEPS, HIDDEN, OWN, W)
            if half == 0:
                out[:, b, :, 0:OWN, :] = hs
            else:
                out[:, b, :, OWN:, :] = hs[:, :, ::-1, :]
    return out

